# revision 1
# baseline (speedup 1.0000x reference)
"""AttentionNet pointer-decoder kernel for 8 Trainium2 NeuronCores.

Strategy (per sharding hint): pure data-parallel over batch. B=512 is split
into 8 shards of 64 batch elements, one per NeuronCore, via jax.pmap; the
small parameter tensors (~0.5 MB) are replicated; memory/mask/tgt are
sharded along batch. No cross-device communication; outputs are gathered
back to the full (512, 1, 2048) shape on the host.

The problem is memory-bound (the 512 MiB `memory` tensor dominates); each
core streams its 64 MiB shard once for the K/V projections and once more
for the pointer projection, which XLA fuses into large matmuls on the PE.
"""

import math

import numpy as np

# Hardcoded problem shape (self-contained; must match the generator).
D = 128
H = 4
DK = D // H
DFF = 512
B = 512
G = 2048
NQ = 1
NEG = -1e9
N_CORES = 8

_COMPILED = {"fn": None, "mode": None}


def _build(jax, jnp, n_dev):
    """Build the pmapped per-shard computation."""

    def layer_norm(x, w, b, eps=1e-5):
        mu = jnp.mean(x, axis=-1, keepdims=True)
        var = jnp.mean((x - mu) ** 2, axis=-1, keepdims=True)
        return (x - mu) / jnp.sqrt(var + eps) * w + b

    def shard_fn(tgt, memory, mask, ln1_w, ln1_b, ln2_w, ln2_b,
                 wq, wk, wv, wo, ffn_w1, ffn_b1, ffn_w2, ffn_b2,
                 ptr_wq, ptr_wk):
        # ---- DecoderLayer ----
        h0 = tgt
        tgt_n = layer_norm(tgt, ln1_w, ln1_b)          # (b, 1, D)
        mem_n = layer_norm(memory, ln1_w, ln1_b)       # (b, G, D)

        norm_factor = 1.0 / math.sqrt(DK)
        Q = jnp.einsum('bnd,hdk->hbnk', tgt_n, wq)
        K = jnp.einsum('bgd,hdk->hbgk', mem_n, wk)
        V = jnp.einsum('bgd,hdk->hbgk', mem_n, wv)
        U = norm_factor * jnp.einsum('hbnk,hbgk->hbng', Q, K)
        m = mask[None]
        U = jnp.where(m, NEG, U)
        attn = jax.nn.softmax(U, axis=-1)
        attn = jnp.where(m, 0.0, attn)
        heads = jnp.einsum('hbng,hbgk->hbnk', attn, V)
        mha_out = jnp.einsum('hbnk,hkd->bnd', heads, wo)

        h = mha_out + h0
        h1 = h
        hn = layer_norm(h, ln2_w, ln2_b)
        ff = jax.nn.relu(hn @ ffn_w1 + ffn_b1) @ ffn_w2 + ffn_b2
        dec = ff + h1

        # ---- SingleHeadAttention pointer ----
        Qp = dec @ ptr_wq
        Kp = memory @ ptr_wk
        Up = (1.0 / math.sqrt(D)) * jnp.einsum('bnd,bgd->bng', Qp, Kp)
        Up = 10.0 * jnp.tanh(Up)
        Up = jnp.where(mask, -10000.0, Up)
        logp = jax.nn.log_softmax(Up, axis=-1)
        return logp

    # Shard the three big tensors over the leading (device) axis; replicate
    # the 14 parameter tensors.
    in_axes = (0, 0, 0) + (None,) * 14
    return jax.pmap(shard_fn, in_axes=in_axes)


def _get_fn():
    if _COMPILED["fn"] is not None:
        return _COMPILED["fn"], _COMPILED["mode"]
    import jax

    devs = jax.devices()
    if len(devs) >= N_CORES:
        fn = _build(jax, jax.numpy, N_CORES)
        mode = "pmap"
    else:
        # Fallback: single-device jit (still correct).
        import jax.numpy as jnp

        inner = _build(jax, jnp, 1)
        fn = inner
        mode = "pmap1"
    _COMPILED["fn"] = fn
    _COMPILED["mode"] = mode
    return fn, mode


def kernel(**inputs) -> np.ndarray:
    import jax

    tgt = np.ascontiguousarray(np.asarray(inputs["tgt"], dtype=np.float32))
    memory = np.ascontiguousarray(np.asarray(inputs["memory"], dtype=np.float32))
    mask = np.ascontiguousarray(np.asarray(inputs["mask"], dtype=bool))

    params = [
        np.asarray(inputs[k], dtype=np.float32)
        for k in ("ln1_w", "ln1_b", "ln2_w", "ln2_b", "wq", "wk", "wv", "wo",
                  "ffn_w1", "ffn_b1", "ffn_w2", "ffn_b2", "ptr_wq", "ptr_wk")
    ]

    fn, mode = _get_fn()
    n_dev = N_CORES if mode == "pmap" else 1
    bs = B // n_dev

    tgt_s = tgt.reshape(n_dev, bs, NQ, D)
    mem_s = memory.reshape(n_dev, bs, G, D)
    mask_s = mask.reshape(n_dev, bs, NQ, G)

    out = fn(tgt_s, mem_s, mask_s, *params)
    out = np.asarray(out, dtype=np.float32).reshape(B, NQ, G)
    return out
